# revision 33
# baseline (speedup 1.0000x reference)
"""Trainium2 Bass kernel for StyleGAN2-style modulated conv2d (ModConv2D).

Reference computation (per sample b):
    w      = kernel * (style[b] + 1)                 # modulate [3,3,Cin,Cout]
    w      = w / sqrt(sum(w^2, (kh,kw,Cin)) + 1e-8)  # demodulate per Cout
    y[b]   = conv2d_same(x[b], w)

Sharding: data-parallel over batch — 16 samples across 8 NeuronCores,
2 samples per core; the base kernel is replicated.

Device algorithm per core (2 samples):
  - conv as 9-tap accumulated matmuls: psum[cout,pix] += w[t,cin,cout]^T @
    xT[cin, pix+off].  x is held channel-major FLAT ([cin, cc, 64+4096+80]
    bf16) with zero guard rows; horizontal (dx=+-1) taps use column-split
    matmuls (N=504, strided psum out) so row wrap never leaks.
  - x ingest transposes: PE (transpose-matmul + batched DVE eviction) for
    sample 0 (critical at startup), DMA-xbar for sample 1 (hidden under
    sample 0's conv).  Output transposes all run on the DMA xbar, split
    across both HWDGE rings.  Weights are modulated on-chip (per-tap, so the
    first conv group unblocks as the per-tap kernel DMAs land).
  - demod factor d[cout] = rsqrt(sum_cin s^2 * K2 + 1e-8) in fp32 on device
    (K2 = sum_t kernel^2 once per core), applied as a per-partition scale on
    psum eviction (ACT).  Output staged bf16, cast back to fp32 by the
    store DMA (SWDGE).
"""

import numpy as np

B, H, W, CIN, COUT, KH, KW = 16, 64, 64, 256, 256, 3, 3
NCORES = 8
BPC = B // NCORES  # samples per core
T = KH * KW  # 9 taps
HWPIX = H * W  # 4096
PAD0 = 64  # zero pixels before the image
XLEN = PAD0 + HWPIX + 80  # 4240: multiple of 16 so xbar dest strides stay 32B-aligned

# tap order: dx=0 taps first so the first matmul of each psum group writes all
# 512 columns with start=True
TAP_ORDER = [1, 4, 7, 0, 3, 6, 2, 5, 8]

_CACHE = {}
LAST_EXEC_NS = None
LAST_MEAN_EXEC_NS = None


def _build_nc():
    from contextlib import ExitStack

    import concourse.bacc as bacc
    import concourse.bass as bass
    import concourse.mybir as mybir
    import concourse.tile as tile
    from concourse.masks import make_identity

    f32 = mybir.dt.float32
    bf16 = mybir.dt.float16  # fp16: same 1 cyc/row PE rate as bf16, 4x finer mantissa
    AF = mybir.ActivationFunctionType

    nc = bacc.Bacc("TRN2", target_bir_lowering=False, debug=False)

    x_d = nc.dram_tensor("x", [BPC, H, W, CIN], f32, kind="ExternalInput")
    s_d = nc.dram_tensor("style", [BPC, CIN], f32, kind="ExternalInput")
    k_d = nc.dram_tensor("kernel", [KH, KW, CIN, COUT], f32, kind="ExternalInput")
    y_d = nc.dram_tensor("y", [BPC, H, W, COUT], f32, kind="ExternalOutput")

    XB = H * W * CIN  # x/y sample stride (elements)
    KKW = CIN * COUT  # kernel tap stride

    def x_blk_ap(b, t8):
        # [128 pix, 4 sblk, 256 cin] starting at pixel (t8*4)*128
        off = b * XB + t8 * 4 * 128 * CIN
        return bass.AP(x_d, off, [[CIN, 128], [128 * CIN, 4], [1, CIN]])

    def y_blk_ap(b, t8):
        off = b * XB + t8 * 4 * 128 * COUT
        return bass.AP(y_d, off, [[COUT, 128], [128 * COUT, 4], [1, COUT]])

    def k_tap_ap(cc, t):
        # [128 cin, 256 cout] for one tap
        return bass.AP(k_d, t * KKW + cc * 128 * COUT, [[COUT, 128], [1, COUT]])

    with tile.TileContext(nc) as tc, ExitStack() as ctx:
        singles = ctx.enter_context(tc.tile_pool(name="singles", bufs=1))
        tmp_pool = ctx.enter_context(tc.tile_pool(name="tmp", bufs=1))
        wpool = ctx.enter_context(tc.tile_pool(name="wpool", bufs=2))
        dpool = ctx.enter_context(tc.tile_pool(name="dpool", bufs=2))
        srow_pool = ctx.enter_context(tc.tile_pool(name="srow", bufs=2))
        xpool = ctx.enter_context(tc.tile_pool(name="xpool", bufs=2))
        xtpool = ctx.enter_context(tc.tile_pool(name="xt", bufs=2 * 8))
        ospool = ctx.enter_context(tc.tile_pool(name="osb", bufs=6))
        obpool = ctx.enter_context(tc.tile_pool(name="ob", bufs=4))
        pconv = ctx.enter_context(tc.tile_pool(name="pconv", bufs=5, space="PSUM"))
        pxt = ctx.enter_context(tc.tile_pool(name="pxt", bufs=2, space="PSUM"))
        psmall = ctx.enter_context(tc.tile_pool(name="psmall", bufs=1, space="PSUM"))

        # style rows + per-tap kernel loads (conv tap order; the modulated
        # weights gate the conv ramp), alternating HWDGE rings
        srows = []
        for b in range(BPC):
            srow = srow_pool.tile([1, CIN], f32, tag="srow")
            nc.scalar.dma_start(out=srow, in_=s_d.ap()[b : b + 1, :])
            srows.append(srow)
        kbase = singles.tile([128, 2, T, COUT], f32)
        for ti, t in enumerate(TAP_ORDER):
            for cc in range(2):
                eng = nc.sync if (ti * 2 + cc) % 2 == 0 else nc.scalar
                eng.dma_start(out=kbase[:, cc, t], in_=k_tap_ap(cc, t))

        # all x loads (cast fp32->bf16, SWDGE) issued upfront; identity for
        # the PE transposes is built after the first two loads are in flight
        xts = [[None] * 8 for _ in range(BPC)]

        def load_xtmp(b, t8):
            xtmp = xtpool.tile([128, 4, CIN], bf16, tag="xtmp", name=f"xtmp_{b}_{t8}")
            nc.gpsimd.dma_start(out=xtmp, in_=x_blk_ap(b, t8))
            xts[b][t8] = xtmp

        load_xtmp(0, 0)
        load_xtmp(0, 1)
        ident_b = singles.tile([128, 128], bf16)
        make_identity(nc, ident_b)
        for b in range(BPC):
            for t8 in range(8):
                if xts[b][t8] is None:
                    load_xtmp(b, t8)

        ones1 = singles.tile([1, 1], f32)
        nc.vector.memset(ones1, 1.0)
        eps_sb = singles.tile([128, 1], f32)
        nc.vector.memset(eps_sb, 1e-8)

        # K2[cin, cout] = sum_t kernel^2  (once per core)
        k2 = singles.tile([128, 2, COUT], f32)
        for cc in range(2):
            k2tmp = tmp_pool.tile([128, T, COUT], f32)
            nc.vector.tensor_mul(k2tmp, kbase[:, cc], kbase[:, cc])
            nc.vector.reduce_sum(
                out=k2[:, cc],
                in_=k2tmp.rearrange("p t c -> p c t"),
                axis=mybir.AxisListType.X,
            )

        # ---- modulation + demod factors for BOTH samples, upfront ----
        wbs, dsbs = [], []
        for b in range(BPC):
            srow1 = srow_pool.tile([1, CIN], f32, tag="srow1")
            nc.vector.tensor_scalar_add(srow1, srows[b], 1.0)

            smod = dpool.tile([128, 2], f32)  # (style+1) col-major per cc
            s2c = dpool.tile([128, 2], f32)
            for cc in range(2):
                pcol = psmall.tile([128, 1], f32, tag="psmall")
                nc.tensor.matmul(
                    pcol, srow1[:, cc * 128 : (cc + 1) * 128], ones1, start=True, stop=True
                )
                nc.vector.tensor_copy(out=smod[:, cc : cc + 1], in_=pcol)
            nc.vector.tensor_mul(s2c, smod, smod)

            # wb[cin, cc, t, cout] = kernel * (s+1), cast bf16, on ACT, per
            # tap in conv order so the first conv matmuls unblock early
            wb = wpool.tile([128, 2, T, COUT], bf16)
            for t in TAP_ORDER:
                for cc in range(2):
                    nc.scalar.activation(
                        wb[:, cc, t], kbase[:, cc, t], AF.Copy,
                        scale=smod[:, cc : cc + 1],
                    )
            wbs.append(wb)

            # sumsq[cout] = sum_cc s2c^T @ k2 -> [1, 256] -> demod d [128, 2]
            prow = psmall.tile([1, COUT], f32, tag="psmall")
            for cc in range(2):
                nc.tensor.matmul(
                    prow, s2c[:, cc : cc + 1], k2[:, cc], start=(cc == 0), stop=(cc == 1)
                )
            ssq_row = srow_pool.tile([1, COUT], f32, tag="ssq")
            nc.vector.tensor_copy(out=ssq_row, in_=prow)
            sqc = dpool.tile([128, 2], f32)
            for oc in range(2):
                pcol2 = psmall.tile([128, 1], f32, tag="psmall")
                nc.tensor.matmul(
                    pcol2, ssq_row[:, oc * 128 : (oc + 1) * 128], ones1, start=True, stop=True
                )
                nc.scalar.activation(sqc[:, oc : oc + 1], pcol2, AF.Sqrt, bias=eps_sb)
            d_sb = dpool.tile([128, 2], f32)
            nc.vector.reciprocal(d_sb, sqc)
            dsbs.append(d_sb)

        for b in range(BPC):
            wb = wbs[b]
            d_sb = dsbs[b]
            # x, channel-major flat: [128 cin, cc, PAD0 + 4096 + 80] bf16
            xflat = xpool.tile([128, 2, XLEN], bf16)
            nc.vector.memset(xflat[:, :, 0:PAD0], 0.0)
            nc.vector.memset(xflat[:, :, PAD0 + HWPIX : XLEN], 0.0)

            def transpose_block_pe(t8):
                # 8 PE transposes + 2 batched DVE evictions per xtmp
                xtmp = xts[b][t8]
                for cc in range(2):
                    pxt_t = pxt.tile([128, 4, 128], bf16, tag="pxt")
                    for s in range(4):
                        nc.tensor.transpose(
                            pxt_t[:, s, :],
                            xtmp[:, s, cc * 128 : (cc + 1) * 128],
                            ident_b,
                        )
                    nc.vector.tensor_copy(
                        out=xflat[:, cc, PAD0 + 512 * t8 : PAD0 + 512 * (t8 + 1)],
                        in_=pxt_t,
                    )

            transpose_block = transpose_block_pe

            def conv_tile(t8):
                # output pixels p0 .. p0+511, both cout chunks
                ob = obpool.tile([128, 4, COUT], bf16, tag="ob")
                p0 = t8 * 512
                for oc in range(2):
                    ps = pconv.tile([128, 512], f32, tag="pconv")
                    ps_r = ps.rearrange("p (r w) -> p r w", w=64)
                    i = 0
                    for t in TAP_ORDER:
                        dy, dx = t // 3 - 1, t % 3 - 1
                        base = PAD0 + p0 + 64 * dy
                        for cc in range(2):
                            lhsT = wb[:, cc, t, oc * 128 : (oc + 1) * 128]
                            xf = xflat[:, cc]
                            if dx == 0:
                                rhs = xf[:, base : base + 512]
                                out_ap = ps
                            elif dx == -1:
                                rhs = xf[:, base : base + 512].rearrange(
                                    "p (r w) -> p r w", w=64
                                )[:, :, 0:63]
                                out_ap = ps_r[:, :, 1:64]
                            else:  # dx == +1
                                rhs = xf[:, base + 1 : base + 513].rearrange(
                                    "p (r w) -> p r w", w=64
                                )[:, :, 0:63]
                                out_ap = ps_r[:, :, 0:63]
                            nc.tensor.matmul(
                                out_ap, lhsT, rhs, start=(i == 0), stop=(i == 17)
                            )
                            i += 1
                    o_sb = ospool.tile([128, 512], bf16, tag="osb")
                    nc.scalar.activation(o_sb, ps, AF.Copy, scale=d_sb[:, oc : oc + 1])
                    if b == BPC - 1 and t8 == 7:
                        # final tile: PE transpose (reusing the ingest psum
                        # pool, idle by now) — shorter tail than xbar+DGE —
                        # and ship each cout half as soon as it is ready
                        pot_t = pxt.tile([128, 4, 128], bf16, tag="pxt")
                        for s in range(4):
                            nc.tensor.transpose(
                                pot_t[:, s, :], o_sb[:, s * 128 : (s + 1) * 128], ident_b
                            )
                        nc.vector.tensor_copy(
                            out=ob[:, :, oc * 128 : (oc + 1) * 128], in_=pot_t
                        )
                        yb = y_blk_ap(b, t8)
                        half = bass.AP(
                            yb.tensor,
                            yb.offset + oc * 128,
                            [[COUT, 128], [128 * COUT, 4], [1, 128]],
                        )
                        nc.gpsimd.dma_start(
                            out=half, in_=ob[:, :, oc * 128 : (oc + 1) * 128]
                        )
                    else:
                        # output transpose on the DMA xbar, split across rings
                        eng = nc.sync if oc == 0 else nc.scalar
                        eng.dma_start_transpose(
                            out=ob[:, :, oc * 128 : (oc + 1) * 128], in_=o_sb
                        )
                if not (b == BPC - 1 and t8 == 7):
                    nc.gpsimd.dma_start(out=y_blk_ap(b, t8), in_=ob)

            PF = 2  # transpose prefetch distance ahead of conv
            for t8 in range(PF):
                transpose_block(t8)
            for t8 in range(PF, 8):
                transpose_block(t8)
                conv_tile(t8 - PF)
            for t8 in range(8 - PF, 8):
                conv_tile(t8)

    nc.compile()
    return nc


def _get_nc():
    if "nc" not in _CACHE:
        _CACHE["nc"] = _build_nc()
    return _CACHE["nc"]


def kernel(x, style, kernel, _trace=False):
    global LAST_EXEC_NS, LAST_MEAN_EXEC_NS
    from concourse.bass_utils import run_bass_kernel_spmd

    x = np.ascontiguousarray(x, dtype=np.float32)
    style = np.ascontiguousarray(style, dtype=np.float32)
    kern = np.ascontiguousarray(kernel, dtype=np.float32)

    nc = _get_nc()
    in_maps = [
        {
            "x": x[i * BPC : (i + 1) * BPC],
            "style": style[i * BPC : (i + 1) * BPC],
            "kernel": kern,
        }
        for i in range(NCORES)
    ]
    res = run_bass_kernel_spmd(nc, in_maps, core_ids=list(range(NCORES)), trace=_trace)
    LAST_EXEC_NS = res.exec_time_ns
    LAST_MEAN_EXEC_NS = res.mean_exec_time_ns
    return np.concatenate([res.results[i]["y"] for i in range(NCORES)], axis=0)


# revision 34
# speedup vs baseline: 1.0049x; 1.0049x over previous
"""Trainium2 Bass kernel for StyleGAN2-style modulated conv2d (ModConv2D).

Reference computation (per sample b):
    w      = kernel * (style[b] + 1)                 # modulate [3,3,Cin,Cout]
    w      = w / sqrt(sum(w^2, (kh,kw,Cin)) + 1e-8)  # demodulate per Cout
    y[b]   = conv2d_same(x[b], w)

Sharding: data-parallel over batch — 16 samples across 8 NeuronCores,
2 samples per core; the base kernel is replicated.

Device algorithm per core (2 samples):
  - conv as 9-tap accumulated matmuls: psum[cout,pix] += w[t,cin,cout]^T @
    xT[cin, pix+off].  x is held channel-major FLAT ([cin, cc, 64+4096+80]
    fp16) with zero guard rows; horizontal (dx=+-1) taps use column-split
    matmuls (N=504, strided psum out) so row wrap never leaks.
  - x ingest transposes: PE (transpose-matmul + batched DVE eviction) for
    sample 0 (critical at startup), DMA-xbar for sample 1 (hidden under
    sample 0's conv).  Output transposes all run on the DMA xbar, split
    across both HWDGE rings.  Weights are modulated on-chip (per-tap, so the
    first conv group unblocks as the per-tap kernel DMAs land).
  - demod factor d[cout] = rsqrt(sum_cin s^2 * K2 + 1e-8) in fp32 on device
    (K2 = sum_t kernel^2 once per core), applied as a per-partition scale on
    psum eviction (ACT).  Output staged fp16, cast back to fp32 by the
    store DMA (SWDGE).
"""

import numpy as np

B, H, W, CIN, COUT, KH, KW = 16, 64, 64, 256, 256, 3, 3
NCORES = 8
BPC = B // NCORES  # samples per core
T = KH * KW  # 9 taps
HWPIX = H * W  # 4096
PAD0 = 64  # zero pixels before the image
XLEN = PAD0 + HWPIX + 80  # 4240: multiple of 16 so xbar dest strides stay 32B-aligned

# tap order: dx=0 taps first so the first matmul of each psum group writes all
# 512 columns with start=True
TAP_ORDER = [1, 4, 7, 0, 3, 6, 2, 5, 8]

_CACHE = {}
LAST_EXEC_NS = None
LAST_MEAN_EXEC_NS = None


def _build_nc():
    from contextlib import ExitStack

    import concourse.bacc as bacc
    import concourse.bass as bass
    import concourse.mybir as mybir
    import concourse.tile as tile
    from concourse.masks import make_identity

    f32 = mybir.dt.float32
    bf16 = mybir.dt.float16  # fp16: same 1 cyc/row PE rate as bf16, 4x finer mantissa
    AF = mybir.ActivationFunctionType

    nc = bacc.Bacc("TRN2", target_bir_lowering=False, debug=False)

    x_d = nc.dram_tensor("x", [BPC, H, W, CIN], f32, kind="ExternalInput")
    s_d = nc.dram_tensor("style", [BPC, CIN], f32, kind="ExternalInput")
    k_d = nc.dram_tensor("kernel", [KH, KW, CIN, COUT], f32, kind="ExternalInput")
    y_d = nc.dram_tensor("y", [BPC, H, W, COUT], f32, kind="ExternalOutput")

    XB = H * W * CIN  # x/y sample stride (elements)
    KKW = CIN * COUT  # kernel tap stride

    def x_blk_ap(b, t8):
        # [128 pix, 4 sblk, 256 cin] starting at pixel (t8*4)*128
        off = b * XB + t8 * 4 * 128 * CIN
        return bass.AP(x_d, off, [[CIN, 128], [128 * CIN, 4], [1, CIN]])

    def y_blk_ap(b, t8):
        off = b * XB + t8 * 4 * 128 * COUT
        return bass.AP(y_d, off, [[COUT, 128], [128 * COUT, 4], [1, COUT]])

    def k_tap_ap(cc, t):
        # [128 cin, 256 cout] for one tap
        return bass.AP(k_d, t * KKW + cc * 128 * COUT, [[COUT, 128], [1, COUT]])

    with tile.TileContext(nc) as tc, ExitStack() as ctx:
        singles = ctx.enter_context(tc.tile_pool(name="singles", bufs=1))
        tmp_pool = ctx.enter_context(tc.tile_pool(name="tmp", bufs=1))
        wpool = ctx.enter_context(tc.tile_pool(name="wpool", bufs=2))
        dpool = ctx.enter_context(tc.tile_pool(name="dpool", bufs=2))
        srow_pool = ctx.enter_context(tc.tile_pool(name="srow", bufs=2))
        xpool = ctx.enter_context(tc.tile_pool(name="xpool", bufs=2))
        xtpool = ctx.enter_context(tc.tile_pool(name="xt", bufs=2 * 8))
        ospool = ctx.enter_context(tc.tile_pool(name="osb", bufs=6))
        obpool = ctx.enter_context(tc.tile_pool(name="ob", bufs=4))
        pconv = ctx.enter_context(tc.tile_pool(name="pconv", bufs=5, space="PSUM"))
        pxt = ctx.enter_context(tc.tile_pool(name="pxt", bufs=2, space="PSUM"))
        psmall = ctx.enter_context(tc.tile_pool(name="psmall", bufs=1, space="PSUM"))

        # style rows + per-tap kernel loads (conv tap order; the modulated
        # weights gate the conv ramp), alternating HWDGE rings
        srows = []
        for b in range(BPC):
            srow = srow_pool.tile([1, CIN], f32, tag="srow")
            nc.scalar.dma_start(out=srow, in_=s_d.ap()[b : b + 1, :])
            srows.append(srow)
        kbase = singles.tile([128, 2, T, COUT], f32)
        for ti, t in enumerate(TAP_ORDER):
            for cc in range(2):
                eng = nc.sync if (ti * 2 + cc) % 2 == 0 else nc.scalar
                eng.dma_start(out=kbase[:, cc, t], in_=k_tap_ap(cc, t))

        # all x loads (cast fp32->fp16, SWDGE) issued upfront; identity for
        # the PE transposes is built after the first two loads are in flight
        xts = [[None] * 8 for _ in range(BPC)]

        def load_xtmp(b, t8):
            xtmp = xtpool.tile([128, 4, CIN], bf16, tag="xtmp", name=f"xtmp_{b}_{t8}")
            nc.gpsimd.dma_start(out=xtmp, in_=x_blk_ap(b, t8))
            xts[b][t8] = xtmp

        load_xtmp(0, 0)
        load_xtmp(0, 1)
        ident_b = singles.tile([128, 128], bf16)
        make_identity(nc, ident_b)
        for b in range(BPC):
            for t8 in range(8):
                if xts[b][t8] is None:
                    load_xtmp(b, t8)

        ones1 = singles.tile([1, 1], f32)
        nc.vector.memset(ones1, 1.0)
        eps_sb = singles.tile([128, 1], f32)
        nc.vector.memset(eps_sb, 1e-8)

        # K2[cin, cout] = sum_t kernel^2  (once per core)
        k2 = singles.tile([128, 2, COUT], f32)
        for cc in range(2):
            k2tmp = tmp_pool.tile([128, T, COUT], f32)
            nc.vector.tensor_mul(k2tmp, kbase[:, cc], kbase[:, cc])
            nc.vector.reduce_sum(
                out=k2[:, cc],
                in_=k2tmp.rearrange("p t c -> p c t"),
                axis=mybir.AxisListType.X,
            )

        # ---- modulation + demod factors for BOTH samples, upfront ----
        wbs, dsbs = [], []
        for b in range(BPC):
            srow1 = srow_pool.tile([1, CIN], f32, tag="srow1")
            nc.vector.tensor_scalar_add(srow1, srows[b], 1.0)

            smod = dpool.tile([128, 2], f32)  # (style+1) col-major per cc
            s2c = dpool.tile([128, 2], f32)
            for cc in range(2):
                pcol = psmall.tile([128, 1], f32, tag="psmall")
                nc.tensor.matmul(
                    pcol, srow1[:, cc * 128 : (cc + 1) * 128], ones1, start=True, stop=True
                )
                nc.vector.tensor_copy(out=smod[:, cc : cc + 1], in_=pcol)
            nc.vector.tensor_mul(s2c, smod, smod)

            # wb[cin, cc, t, cout] = kernel * (s+1), cast fp16, on ACT, per
            # tap in conv order so the first conv matmuls unblock early
            wb = wpool.tile([128, 2, T, COUT], bf16)
            for t in TAP_ORDER:
                for cc in range(2):
                    nc.scalar.activation(
                        wb[:, cc, t], kbase[:, cc, t], AF.Copy,
                        scale=smod[:, cc : cc + 1],
                    )
            wbs.append(wb)

            # sumsq[cout] = sum_cc s2c^T @ k2 -> [1, 256] -> demod d [128, 2]
            prow = psmall.tile([1, COUT], f32, tag="psmall")
            for cc in range(2):
                nc.tensor.matmul(
                    prow, s2c[:, cc : cc + 1], k2[:, cc], start=(cc == 0), stop=(cc == 1)
                )
            ssq_row = srow_pool.tile([1, COUT], f32, tag="ssq")
            nc.vector.tensor_copy(out=ssq_row, in_=prow)
            sqc = dpool.tile([128, 2], f32)
            for oc in range(2):
                pcol2 = psmall.tile([128, 1], f32, tag="psmall")
                nc.tensor.matmul(
                    pcol2, ssq_row[:, oc * 128 : (oc + 1) * 128], ones1, start=True, stop=True
                )
                nc.scalar.activation(sqc[:, oc : oc + 1], pcol2, AF.Sqrt, bias=eps_sb)
            d_sb = dpool.tile([128, 2], f32)
            nc.vector.reciprocal(d_sb, sqc)
            dsbs.append(d_sb)

        for b in range(BPC):
            wb = wbs[b]
            d_sb = dsbs[b]
            # x, channel-major flat: [128 cin, cc, PAD0 + 4096 + 80] bf16
            xflat = xpool.tile([128, 2, XLEN], bf16)
            nc.vector.memset(xflat[:, :, 0:PAD0], 0.0)
            nc.vector.memset(xflat[:, :, PAD0 + HWPIX : XLEN], 0.0)

            def transpose_block_pe(t8):
                # 8 PE transposes + 2 batched DVE evictions per xtmp
                xtmp = xts[b][t8]
                for cc in range(2):
                    pxt_t = pxt.tile([128, 4, 128], bf16, tag="pxt")
                    for s in range(4):
                        nc.tensor.transpose(
                            pxt_t[:, s, :],
                            xtmp[:, s, cc * 128 : (cc + 1) * 128],
                            ident_b,
                        )
                    nc.vector.tensor_copy(
                        out=xflat[:, cc, PAD0 + 512 * t8 : PAD0 + 512 * (t8 + 1)],
                        in_=pxt_t,
                    )

            transpose_block = transpose_block_pe

            def conv_tile(t8):
                # output pixels p0 .. p0+511, both cout chunks
                ob = obpool.tile([128, 4, COUT], bf16, tag="ob")
                p0 = t8 * 512
                for oc in range(2):
                    ps = pconv.tile([128, 512], f32, tag="pconv")
                    ps_r = ps.rearrange("p (r w) -> p r w", w=64)
                    i = 0
                    for t in TAP_ORDER:
                        dy, dx = t // 3 - 1, t % 3 - 1
                        base = PAD0 + p0 + 64 * dy
                        for cc in range(2):
                            lhsT = wb[:, cc, t, oc * 128 : (oc + 1) * 128]
                            xf = xflat[:, cc]
                            if dx == 0:
                                rhs = xf[:, base : base + 512]
                                out_ap = ps
                            elif dx == -1:
                                rhs = xf[:, base : base + 512].rearrange(
                                    "p (r w) -> p r w", w=64
                                )[:, :, 0:63]
                                out_ap = ps_r[:, :, 1:64]
                            else:  # dx == +1
                                rhs = xf[:, base + 1 : base + 513].rearrange(
                                    "p (r w) -> p r w", w=64
                                )[:, :, 0:63]
                                out_ap = ps_r[:, :, 0:63]
                            nc.tensor.matmul(
                                out_ap, lhsT, rhs, start=(i == 0), stop=(i == 17)
                            )
                            i += 1
                    o_sb = ospool.tile([128, 512], bf16, tag="osb")
                    nc.scalar.activation(o_sb, ps, AF.Copy, scale=d_sb[:, oc : oc + 1])
                    if b == BPC - 1 and t8 == 7:
                        # final tile: PE transpose (reusing the ingest psum
                        # pool, idle by now) — shorter tail than xbar+DGE —
                        # and ship each cout half as soon as it is ready
                        pot_t = pxt.tile([128, 4, 128], bf16, tag="pxt")
                        for s in range(4):
                            nc.tensor.transpose(
                                pot_t[:, s, :], o_sb[:, s * 128 : (s + 1) * 128], ident_b
                            )
                        nc.vector.tensor_copy(
                            out=ob[:, :, oc * 128 : (oc + 1) * 128], in_=pot_t
                        )
                        yb = y_blk_ap(b, t8)
                        half = bass.AP(
                            yb.tensor,
                            yb.offset + oc * 128,
                            [[COUT, 128], [128 * COUT, 4], [1, 128]],
                        )
                        nc.gpsimd.dma_start(
                            out=half, in_=ob[:, :, oc * 128 : (oc + 1) * 128]
                        )
                    else:
                        # output transpose on the DMA xbar, split across rings
                        eng = nc.sync if oc == 0 else nc.scalar
                        eng.dma_start_transpose(
                            out=ob[:, :, oc * 128 : (oc + 1) * 128], in_=o_sb
                        )
                if not (b == BPC - 1 and t8 == 7):
                    nc.gpsimd.dma_start(out=y_blk_ap(b, t8), in_=ob)

            PF = 2  # transpose prefetch distance ahead of conv
            for t8 in range(PF):
                transpose_block(t8)
            for t8 in range(PF, 8):
                transpose_block(t8)
                conv_tile(t8 - PF)
            for t8 in range(8 - PF, 8):
                conv_tile(t8)

    nc.compile()
    return nc


def _get_nc():
    if "nc" not in _CACHE:
        _CACHE["nc"] = _build_nc()
    return _CACHE["nc"]


def kernel(x, style, kernel, _trace=False):
    global LAST_EXEC_NS, LAST_MEAN_EXEC_NS
    from concourse.bass_utils import run_bass_kernel_spmd

    x = np.ascontiguousarray(x, dtype=np.float32)
    style = np.ascontiguousarray(style, dtype=np.float32)
    kern = np.ascontiguousarray(kernel, dtype=np.float32)

    nc = _get_nc()
    in_maps = [
        {
            "x": x[i * BPC : (i + 1) * BPC],
            "style": style[i * BPC : (i + 1) * BPC],
            "kernel": kern,
        }
        for i in range(NCORES)
    ]
    res = run_bass_kernel_spmd(nc, in_maps, core_ids=list(range(NCORES)), trace=_trace)
    LAST_EXEC_NS = res.exec_time_ns
    LAST_MEAN_EXEC_NS = res.mean_exec_time_ns
    return np.concatenate([res.results[i]["y"] for i in range(NCORES)], axis=0)
